# revision 9
# baseline (speedup 1.0000x reference)
"""CrossModalAttention Trainium2 kernel.

Full-input contract: kernel(**inputs) takes the unsharded inputs from
setup_inputs() and returns (out [4,2048,4096], attn_mean [4,2048,576]) as fp32.

Sharding (8 NeuronCores): 4-way data parallel over batch x 2-way tensor
parallel over heads (Megatron): column-parallel Q/K/V projections (each shard
owns 8 of 16 heads = 2048 of 4096 projected features), row-parallel out_proj
(host sums the two partial outputs per batch), and the head-mean attention
weights are summed across the two head shards on the host.

Device kernel (identical SPMD program, per-core data differs):
  inputs (per core, fp32):
    qT [4096,2048]  = query[b].T          wq/wk/wv [4096,2048] column slices
    kT [4096, 576]  = key[b].T            wo [2048,4096] row slice
    vT [4096, 576]  = value[b].T          bq/bk/bv [2048] slices, bo2 [4096]=bo/2
  outputs:
    outT_p [4096,2048]  = (ctx @ wo + bo/2).T   (partial over head shards)
    attn_p [8,576,2048] = per-local-head normalized attention (S^T layout)

All matmuls run as float32r (full PE rate for free-dim >= 256, fp32 storage).
Activations flow "feature-major" (transposed) so the contraction dim is always
on SBUF partitions; softmax runs on S^T tiles with column sums computed by
ones-vector matmuls on the PE and broadcast back via a rank-1 matmul.
"""

import numpy as np

N_CORES = 8
B = 4
SQ = 2048
SK = 576
D = 4096
H = 16
HD = 256
HLOC = 8          # heads per shard
DLOC = 2048       # projected features per shard
KC = D // 128     # 32 contraction chunks
SCALE = HD ** -0.5

_BUILD_CACHE = {}


def _build(scale: float):
    """Build + compile the per-core Bass program. scale = SCALE / temperature."""
    if scale in _BUILD_CACHE:
        return _BUILD_CACHE[scale]

    from contextlib import ExitStack

    import concourse.bacc as bacc
    import concourse.tile as tile
    from concourse import mybir
    from concourse.masks import make_identity

    f32 = mybir.dt.float32
    f32r = mybir.dt.float32r
    IDENT = mybir.ActivationFunctionType.Identity
    EXP = mybir.ActivationFunctionType.Exp

    nc = bacc.Bacc("TRN2", target_bir_lowering=False, debug=False,
                   enable_asserts=True, num_devices=1)

    qT = nc.dram_tensor("qT", [D, SQ], f32r, kind="ExternalInput").ap()
    kT = nc.dram_tensor("kT", [D, SK], f32r, kind="ExternalInput").ap()
    vT = nc.dram_tensor("vT", [D, SK], f32r, kind="ExternalInput").ap()
    wq = nc.dram_tensor("wq", [D, DLOC], f32r, kind="ExternalInput").ap()
    wk = nc.dram_tensor("wk", [D, DLOC], f32r, kind="ExternalInput").ap()
    wv = nc.dram_tensor("wv", [D, DLOC], f32r, kind="ExternalInput").ap()
    wo = nc.dram_tensor("wo", [DLOC, D], f32r, kind="ExternalInput").ap()
    bq = nc.dram_tensor("bq", [DLOC], f32, kind="ExternalInput").ap()
    bk = nc.dram_tensor("bk", [DLOC], f32, kind="ExternalInput").ap()
    bv = nc.dram_tensor("bv", [DLOC], f32, kind="ExternalInput").ap()
    bo2 = nc.dram_tensor("bo2", [D], f32, kind="ExternalInput").ap()

    outT_p = nc.dram_tensor("outT_p", [D, SQ], f32, kind="ExternalOutput").ap()
    attn_p = nc.dram_tensor("attn_p", [HLOC, SK, SQ], f32, kind="ExternalOutput").ap()

    # DRAM scratch for inter-phase spills
    KT_d = nc.dram_tensor("KT_d", [DLOC, SK], f32r, kind="Internal").ap()
    V_d = nc.dram_tensor("V_d", [SK, DLOC], f32r, kind="Internal").ap()
    QT_d = nc.dram_tensor("QT_d", [DLOC, SQ], f32r, kind="Internal").ap()

    MC = DLOC // 128  # 16 output-feature chunks per shard

    with tile.TileContext(nc) as tc:
        with ExitStack() as top:
            const = top.enter_context(tc.tile_pool(name="const", bufs=1))
            ident_f = const.tile([128, 128], f32)
            make_identity(nc, ident_f)
            ident = const.tile([128, 128], f32r)
            nc.scalar.copy(ident[:], ident_f[:])
            ones_f = const.tile([128, 1], f32)
            nc.vector.memset(ones_f[:], 1.0)
            ones_col = const.tile([128, 1], f32r)
            nc.scalar.copy(ones_col[:], ones_f[:])
            ones_rf = const.tile([1, 128], f32)
            nc.vector.memset(ones_rf[:], 1.0)
            ones_row = const.tile([1, 128], f32r)
            nc.scalar.copy(ones_row[:], ones_rf[:])
            bq_sb = const.tile([128, MC], f32)
            nc.sync.dma_start(bq_sb[:], bq.rearrange("(m p) -> p m", p=128))
            bk_sb = const.tile([128, MC], f32)
            nc.sync.dma_start(bk_sb[:], bk.rearrange("(m p) -> p m", p=128))
            bv_sb = const.tile([128, MC], f32)
            nc.sync.dma_start(bv_sb[:], bv.rearrange("(m p) -> p m", p=128))
            bo2_sb = const.tile([128, KC], f32)
            nc.sync.dma_start(bo2_sb[:], bo2.rearrange("(m p) -> p m", p=128))

            # ---------------- Phase A1: KT_d = (k @ wk + bk).T ----------------
            with ExitStack() as ctx:
                ktp = ctx.enter_context(tc.tile_pool(name="ktp", bufs=1))
                wkp = ctx.enter_context(tc.tile_pool(name="wkp", bufs=2))
                kout = ctx.enter_context(tc.tile_pool(name="kout", bufs=3))
                psA = ctx.enter_context(tc.tile_pool(name="psA", bufs=2, space="PSUM"))

                kt_all = ktp.tile([128, KC * SK], f32r)
                nc.sync.dma_start(kt_all[:].rearrange("p (kc s) -> p kc s", kc=KC),
                              kT.rearrange("(kc p) s -> p kc s", p=128))
                for mc in range(MC):
                    wkc = wkp.tile([128, KC * 128], f32r, tag="wkc")
                    nc.sync.dma_start(
                        wkc[:].rearrange("p (kc m) -> p kc m", kc=KC),
                        wk[:, mc * 128:(mc + 1) * 128]
                        .rearrange("(kc p) m -> p kc m", p=128))
                    ps = psA.tile([128, 1024], f32, tag="psA")
                    for kc in range(KC):
                        lhs = wkc[:, kc * 128:(kc + 1) * 128]
                        for g0, g1, po in ((0, 288, 0), (288, 576, 512)):
                            nc.tensor.matmul(
                                ps[:, po:po + (g1 - g0)], lhs,
                                kt_all[:, kc * SK + g0: kc * SK + g1],
                                start=(kc == 0), stop=(kc == KC - 1))
                    ko = kout.tile([128, SK], f32r, tag="ko")
                    nc.scalar.activation(ko[:, 0:288], ps[:, 0:288], IDENT,
                                         bias=bk_sb[:, mc:mc + 1])
                    nc.scalar.activation(ko[:, 288:576], ps[:, 512:800], IDENT,
                                         bias=bk_sb[:, mc:mc + 1])
                    nc.sync.dma_start(KT_d[mc * 128:(mc + 1) * 128, :], ko[:])

            # ------------- Phase A2: V_d = v @ wv + bv (via VT + transpose) -------------
            with ExitStack() as ctx:
                vtp = ctx.enter_context(tc.tile_pool(name="vtp", bufs=1))
                wvp = ctx.enter_context(tc.tile_pool(name="wvp", bufs=2))
                vout = ctx.enter_context(tc.tile_pool(name="vout", bufs=2))
                vev = ctx.enter_context(tc.tile_pool(name="vev", bufs=4))
                psA2 = ctx.enter_context(tc.tile_pool(name="psA2", bufs=2, space="PSUM"))
                psT = ctx.enter_context(tc.tile_pool(name="psT", bufs=4, space="PSUM"))

                vt_all = vtp.tile([128, KC * SK], f32r)
                nc.sync.dma_start(vt_all[:].rearrange("p (kc s) -> p kc s", kc=KC),
                              vT.rearrange("(kc p) s -> p kc s", p=128))
                for mc in range(MC):
                    wvc = wvp.tile([128, KC * 128], f32r, tag="wvc")
                    nc.sync.dma_start(
                        wvc[:].rearrange("p (kc m) -> p kc m", kc=KC),
                        wv[:, mc * 128:(mc + 1) * 128]
                        .rearrange("(kc p) m -> p kc m", p=128))
                    ps = psA2.tile([128, 1024], f32, tag="psA2")
                    for kc in range(KC):
                        lhs = wvc[:, kc * 128:(kc + 1) * 128]
                        for g0, g1, po in ((0, 288, 0), (288, 576, 512)):
                            nc.tensor.matmul(
                                ps[:, po:po + (g1 - g0)], lhs,
                                vt_all[:, kc * SK + g0: kc * SK + g1],
                                start=(kc == 0), stop=(kc == KC - 1))
                    vo = vout.tile([128, SK], f32r, tag="vo")
                    nc.scalar.activation(vo[:, 0:288], ps[:, 0:288], IDENT,
                                         bias=bv_sb[:, mc:mc + 1])
                    nc.scalar.activation(vo[:, 288:576], ps[:, 512:800], IDENT,
                                         bias=bv_sb[:, mc:mc + 1])
                    # transpose VT tile [128, 576] -> V chunks [<=128, 128]
                    for c5 in range(5):
                        sz = 128 if c5 < 4 else 64
                        pt = psT.tile([128, 128], f32r, tag="pt")
                        nc.tensor.transpose(
                            pt[:sz, :], vo[:, c5 * 128: c5 * 128 + sz], ident[:])
                        ve = vev.tile([128, 128], f32r, tag="ve")
                        nc.scalar.copy(ve[:sz, :], pt[:sz, :])
                        nc.sync.dma_start(
                            V_d[c5 * 128: c5 * 128 + sz, mc * 128:(mc + 1) * 128],
                            ve[:sz, :])

            # ---------------- Phase B: QT_d = (q @ wq + bq).T ----------------
            WSTRIP = 1024
            with ExitStack() as ctx:
                qtp = ctx.enter_context(tc.tile_pool(name="qtp", bufs=1))
                wqp = ctx.enter_context(tc.tile_pool(name="wqp", bufs=2))
                qout = ctx.enter_context(tc.tile_pool(name="qout", bufs=3))
                psB = ctx.enter_context(tc.tile_pool(name="psB", bufs=2, space="PSUM"))

                for w in range(SQ // WSTRIP):
                    qts = qtp.tile([128, KC * WSTRIP], f32r, tag="qts")
                    nc.sync.dma_start(
                        qts[:].rearrange("p (kc s) -> p kc s", kc=KC),
                        qT[:, w * WSTRIP:(w + 1) * WSTRIP]
                        .rearrange("(kc p) s -> p kc s", p=128))
                    for mc in range(MC):
                        wqc = wqp.tile([128, KC * 128], f32r, tag="wqc")
                        nc.sync.dma_start(
                            wqc[:].rearrange("p (kc m) -> p kc m", kc=KC),
                            wq[:, mc * 128:(mc + 1) * 128]
                            .rearrange("(kc p) m -> p kc m", p=128))
                        ps = psB.tile([128, WSTRIP], f32, tag="psB")
                        for kc in range(KC):
                            lhs = wqc[:, kc * 128:(kc + 1) * 128]
                            for g in range(WSTRIP // 512):
                                nc.tensor.matmul(
                                    ps[:, g * 512:(g + 1) * 512], lhs,
                                    qts[:, kc * WSTRIP + g * 512:
                                        kc * WSTRIP + (g + 1) * 512],
                                    start=(kc == 0), stop=(kc == KC - 1))
                        qo = qout.tile([128, WSTRIP], f32r, tag="qo")
                        nc.scalar.activation(qo[:], ps[:], IDENT,
                                             bias=bq_sb[:, mc:mc + 1])
                        nc.sync.dma_start(
                            QT_d[mc * 128:(mc + 1) * 128,
                                 w * WSTRIP:(w + 1) * WSTRIP], qo[:])

            # ---------------- Phase C: attention per local head ----------------
            # ctxT stays resident across C and D: [128, 16*2048] = 128 KB/part
            ctxp = top.enter_context(tc.tile_pool(name="ctxp", bufs=1))
            ctx_all = ctxp.tile([128, MC * SQ], f32r)

            KEY_CH = [(0, 128), (128, 256), (256, 384), (384, 512), (512, 576)]
            with ExitStack() as ctx:
                khp = ctx.enter_context(tc.tile_pool(name="khp", bufs=2))
                vhp = ctx.enter_context(tc.tile_pool(name="vhp", bufs=2))
                qhp = ctx.enter_context(tc.tile_pool(name="qhp", bufs=3))
                stp = ctx.enter_context(tc.tile_pool(name="stp", bufs=2))
                rbp = ctx.enter_context(tc.tile_pool(name="rbp", bufs=2))
                rvp = ctx.enter_context(tc.tile_pool(name="rvp", bufs=2))
                psST = ctx.enter_context(tc.tile_pool(name="psST", bufs=2, space="PSUM"))
                psR = ctx.enter_context(tc.tile_pool(name="psR", bufs=1, space="PSUM"))
                psBC = ctx.enter_context(tc.tile_pool(name="psBC", bufs=1, space="PSUM"))
                psCX = ctx.enter_context(tc.tile_pool(name="psCX", bufs=2, space="PSUM"))

                for h in range(HLOC):
                    kth = khp.tile([128, 2 * SK], f32r, tag="kth")
                    nc.sync.dma_start(
                        kth[:].rearrange("p (c s) -> p c s", c=2),
                        KT_d[h * HD:(h + 1) * HD, :]
                        .rearrange("(c p) s -> p c s", p=128))
                    vha = vhp.tile([128, 4 * HD], f32r, tag="vha")
                    nc.sync.dma_start(
                        vha[:].rearrange("p (c d) -> p c d", c=4),
                        V_d[0:512, h * HD:(h + 1) * HD]
                        .rearrange("(c p) d -> p c d", p=128))
                    vhb = vhp.tile([64, HD], f32r, tag="vhb")
                    nc.sync.dma_start(vhb[:], V_d[512:SK, h * HD:(h + 1) * HD])

                    for rg in range(SQ // 512):
                        qth = qhp.tile([128, 2 * 512], f32r, tag="qth")
                        nc.sync.dma_start(
                            qth[:].rearrange("p (c s) -> p c s", c=2),
                            QT_d[h * HD:(h + 1) * HD, rg * 512:(rg + 1) * 512]
                            .rearrange("(c p) s -> p c s", p=128))

                        stn = stp.tile([128, 5 * 512], f32r, tag="stn")
                        for c5, (k0, k1) in enumerate(KEY_CH):
                            sz = k1 - k0
                            pst = psST.tile([128, 512], f32, tag="pst")
                            for c2 in range(2):
                                nc.tensor.matmul(
                                    pst[:sz, :],
                                    kth[:, c2 * SK + k0: c2 * SK + k1],
                                    qth[:, c2 * 512:(c2 + 1) * 512],
                                    start=(c2 == 0), stop=(c2 == 1))
                            nc.scalar.activation(
                                stn[:sz, c5 * 512:(c5 + 1) * 512], pst[:sz, :],
                                EXP, scale=scale)

                        # column sums over keys via ones-matmul, then 1/x
                        psr = psR.tile([1, 512], f32, tag="psr")
                        for c5, (k0, k1) in enumerate(KEY_CH):
                            sz = k1 - k0
                            nc.tensor.matmul(
                                psr[:, :], ones_col[:sz, :],
                                stn[:sz, c5 * 512:(c5 + 1) * 512],
                                start=(c5 == 0), stop=(c5 == 4))
                        rinv_f = rvp.tile([1, 512], f32, tag="rinv_f")
                        nc.vector.reciprocal(rinv_f[:], psr[:, :])
                        rinv = rvp.tile([1, 512], f32r, tag="rinv")
                        nc.scalar.copy(rinv[:], rinv_f[:])
                        # broadcast rinv across 128 partitions: ones [1,128]^T @ rinv
                        psb = psBC.tile([128, 512], f32, tag="psb")
                        nc.tensor.matmul(psb[:], ones_row[:],
                                         rinv[:],
                                         start=True, stop=True)
                        rb = rbp.tile([128, 512], f32, tag="rb")
                        nc.scalar.copy(rb[:], psb[:])

                        for c5, (k0, k1) in enumerate(KEY_CH):
                            sz = k1 - k0
                            sl = stn[:sz, c5 * 512:(c5 + 1) * 512]
                            nc.vector.tensor_mul(sl, sl, rb[:sz, :])
                            nc.sync.dma_start(
                                attn_p[h, k0:k1, rg * 512:(rg + 1) * 512],
                                sl.bitcast(f32))

                        for c2 in range(2):
                            pcx = psCX.tile([128, 512], f32, tag="pcx")
                            for c5, (k0, k1) in enumerate(KEY_CH):
                                sz = k1 - k0
                                if c5 < 4:
                                    vsl = vha[:, c5 * HD + c2 * 128:
                                              c5 * HD + (c2 + 1) * 128]
                                else:
                                    vsl = vhb[:, c2 * 128:(c2 + 1) * 128]
                                nc.tensor.matmul(
                                    pcx[:], vsl,
                                    stn[:sz, c5 * 512:(c5 + 1) * 512],
                                    start=(c5 == 0), stop=(c5 == 4))
                            nc.scalar.copy(
                                ctx_all[:, (h * 2 + c2) * SQ + rg * 512:
                                        (h * 2 + c2) * SQ + (rg + 1) * 512],
                                pcx[:])

            # ---------------- Phase D: outT_p = wo.T @ ctxT + bo/2 ----------------
            with ExitStack() as ctx:
                wop = ctx.enter_context(tc.tile_pool(name="wop", bufs=2))
                opool = ctx.enter_context(tc.tile_pool(name="opool", bufs=4))
                psD = ctx.enter_context(tc.tile_pool(name="psD", bufs=2, space="PSUM"))

                for oc in range(D // 128):
                    woc = wop.tile([128, MC * 128], f32r, tag="woc")
                    nc.sync.dma_start(
                        woc[:].rearrange("p (fc m) -> p fc m", fc=MC),
                        wo[:, oc * 128:(oc + 1) * 128]
                        .rearrange("(fc p) m -> p fc m", p=128))
                    pss = [psD.tile([128, 512], f32, tag=f"psD{rg}",
                                    name=f"psD{rg}") for rg in range(4)]
                    for fc in range(MC):
                        lhs = woc[:, fc * 128:(fc + 1) * 128]
                        for rg in range(4):
                            nc.tensor.matmul(
                                pss[rg][:], lhs,
                                ctx_all[:, fc * SQ + rg * 512:
                                        fc * SQ + (rg + 1) * 512],
                                start=(fc == 0), stop=(fc == MC - 1))
                    for rg in range(4):
                        osb = opool.tile([128, 512], f32, tag="osb")
                        nc.scalar.activation(osb[:], pss[rg][:], IDENT,
                                             bias=bo2_sb[:, oc:oc + 1])
                        nc.sync.dma_start(
                            outT_p[oc * 128:(oc + 1) * 128,
                                   rg * 512:(rg + 1) * 512], osb[:])

    nc.compile()
    _BUILD_CACHE[scale] = nc
    return nc


def _make_in_maps(query, key, value, Wq, bq, Wk, bk, Wv, bv, Wo, bo):
    f = np.float32
    bo2 = (np.asarray(bo, f) / 2.0).astype(f)
    in_maps = []
    for c in range(N_CORES):
        b, s = c // 2, c % 2
        sl = slice(s * DLOC, (s + 1) * DLOC)
        in_maps.append({
            "qT": np.ascontiguousarray(np.asarray(query[b], f).T),
            "kT": np.ascontiguousarray(np.asarray(key[b], f).T),
            "vT": np.ascontiguousarray(np.asarray(value[b], f).T),
            "wq": np.ascontiguousarray(np.asarray(Wq, f)[:, sl]),
            "wk": np.ascontiguousarray(np.asarray(Wk, f)[:, sl]),
            "wv": np.ascontiguousarray(np.asarray(Wv, f)[:, sl]),
            "wo": np.ascontiguousarray(np.asarray(Wo, f)[sl, :]),
            "bq": np.ascontiguousarray(np.asarray(bq, f)[sl]),
            "bk": np.ascontiguousarray(np.asarray(bk, f)[sl]),
            "bv": np.ascontiguousarray(np.asarray(bv, f)[sl]),
            "bo2": bo2,
        })
    return in_maps


def _gather(results):
    out = np.empty((B, SQ, D), np.float32)
    attn = np.empty((B, SQ, SK), np.float32)
    for b in range(B):
        r0, r1 = results[2 * b], results[2 * b + 1]
        out[b] = (r0["outT_p"] + r1["outT_p"]).T
        attn[b] = ((r0["attn_p"].sum(axis=0) + r1["attn_p"].sum(axis=0)) / H).T
    return out, attn


def kernel(query, key, value, Wq, bq, Wk, bk, Wv, bv, Wo, bo, temperature):
    from concourse.bass_utils import run_bass_kernel_spmd

    temp = float(np.asarray(temperature))
    nc = _build(SCALE / temp)
    in_maps = _make_in_maps(query, key, value, Wq, bq, Wk, bk, Wv, bv, Wo, bo)
    res = run_bass_kernel_spmd(nc, in_maps, core_ids=list(range(N_CORES)))
    return _gather(res.results)


# revision 13
# speedup vs baseline: 1.1275x; 1.1275x over previous
"""CrossModalAttention Trainium2 kernel.

Full-input contract: kernel(**inputs) takes the unsharded inputs from
setup_inputs() and returns (out [4,2048,4096], attn_mean [4,2048,576]) as fp32.

Sharding (8 NeuronCores): 4-way data parallel over batch x 2-way tensor
parallel over heads (Megatron): column-parallel Q/K/V projections (each shard
owns 8 of 16 heads = 2048 of 4096 projected features), row-parallel out_proj
(host sums the two partial outputs per batch), and the head-mean attention
weights are summed across the two head shards on the host.

Device kernel (identical SPMD program, per-core data differs):
  inputs (per core, fp32):
    qT [4096,2048]  = query[b].T          wq/wk/wv [4096,2048] column slices
    kT [4096, 576]  = key[b].T            wo [2048,4096] row slice
    vT [4096, 576]  = value[b].T          bq/bk/bv [2048] slices, bo2 [4096]=bo/2
  outputs:
    outT_p [4096,2048]  = (ctx @ wo + bo/2).T   (partial over head shards)
    attn_p [8,576,2048] = per-local-head normalized attention (S^T layout)

All matmuls run as float32r (full PE rate for free-dim >= 256, fp32 storage).
Activations flow "feature-major" (transposed) so the contraction dim is always
on SBUF partitions; softmax runs on S^T tiles with column sums computed by
ones-vector matmuls on the PE and broadcast back via a rank-1 matmul.
"""

import numpy as np

N_CORES = 8
B = 4
SQ = 2048
SK = 576
D = 4096
H = 16
HD = 256
HLOC = 8          # heads per shard
DLOC = 2048       # projected features per shard
KC = D // 128     # 32 contraction chunks
SCALE = HD ** -0.5

_BUILD_CACHE = {}


def _build(scale: float, phases: str = "ABCD"):
    """Build + compile the per-core Bass program. scale = SCALE / temperature."""
    key = (scale, phases)
    if key in _BUILD_CACHE:
        return _BUILD_CACHE[key]

    from contextlib import ExitStack

    import concourse.bacc as bacc
    import concourse.tile as tile
    from concourse import mybir
    from concourse.masks import make_identity

    f32 = mybir.dt.float32
    f32r = mybir.dt.float32r
    IDENT = mybir.ActivationFunctionType.Identity
    EXP = mybir.ActivationFunctionType.Exp

    nc = bacc.Bacc("TRN2", target_bir_lowering=False, debug=False,
                   enable_asserts=True, num_devices=1)

    qT = nc.dram_tensor("qT", [D, SQ], f32r, kind="ExternalInput").ap()
    kT = nc.dram_tensor("kT", [D, SK], f32r, kind="ExternalInput").ap()
    vT = nc.dram_tensor("vT", [D, SK], f32r, kind="ExternalInput").ap()
    wq = nc.dram_tensor("wq", [D, DLOC], f32r, kind="ExternalInput").ap()
    wk = nc.dram_tensor("wk", [D, DLOC], f32r, kind="ExternalInput").ap()
    wv = nc.dram_tensor("wv", [D, DLOC], f32r, kind="ExternalInput").ap()
    wo = nc.dram_tensor("wo", [DLOC, D], f32r, kind="ExternalInput").ap()
    bq = nc.dram_tensor("bq", [DLOC], f32, kind="ExternalInput").ap()
    bk = nc.dram_tensor("bk", [DLOC], f32, kind="ExternalInput").ap()
    bv = nc.dram_tensor("bv", [DLOC], f32, kind="ExternalInput").ap()
    bo2 = nc.dram_tensor("bo2", [D], f32, kind="ExternalInput").ap()

    outT_p = nc.dram_tensor("outT_p", [D, SQ], f32, kind="ExternalOutput").ap()
    attn_p = nc.dram_tensor("attn_p", [HLOC, SK, SQ], f32, kind="ExternalOutput").ap()

    # DRAM scratch for inter-phase spills
    KT_d = nc.dram_tensor("KT_d", [DLOC, SK], f32r, kind="Internal").ap()
    V_d = nc.dram_tensor("V_d", [SK, DLOC], f32r, kind="Internal").ap()
    QT_d = nc.dram_tensor("QT_d", [DLOC, SQ], f32r, kind="Internal").ap()

    MC = DLOC // 128  # 16 output-feature chunks per shard

    with tile.TileContext(nc) as tc:
        with ExitStack() as top:
            const = top.enter_context(tc.tile_pool(name="const", bufs=1))
            ident_f = const.tile([128, 128], f32)
            make_identity(nc, ident_f)
            ident = const.tile([128, 128], f32r)
            nc.scalar.copy(ident[:], ident_f[:])
            ones_f = const.tile([128, 1], f32)
            nc.vector.memset(ones_f[:], 1.0)
            ones_col = const.tile([128, 1], f32r)
            nc.scalar.copy(ones_col[:], ones_f[:])
            ones_rf = const.tile([1, 128], f32)
            nc.vector.memset(ones_rf[:], 1.0)
            ones_row = const.tile([1, 128], f32r)
            nc.scalar.copy(ones_row[:], ones_rf[:])
            bq_sb = const.tile([128, MC], f32)
            nc.sync.dma_start(bq_sb[:], bq.rearrange("(m p) -> p m", p=128))
            bk_sb = const.tile([128, MC], f32)
            nc.sync.dma_start(bk_sb[:], bk.rearrange("(m p) -> p m", p=128))
            bv_sb = const.tile([128, MC], f32)
            nc.sync.dma_start(bv_sb[:], bv.rearrange("(m p) -> p m", p=128))
            bo2_sb = const.tile([128, KC], f32)
            nc.sync.dma_start(bo2_sb[:], bo2.rearrange("(m p) -> p m", p=128))

            # ------- Phase A: KT_d = (k@wk+bk).T and V_d = v@wv+bv (interleaved) -------
            with ExitStack() as ctx:
                ktp = ctx.enter_context(tc.tile_pool(name="ktp", bufs=1))
                vtp = ctx.enter_context(tc.tile_pool(name="vtp", bufs=1))
                wkp = ctx.enter_context(tc.tile_pool(name="wkp", bufs=2))
                kout = ctx.enter_context(tc.tile_pool(name="kout", bufs=2))
                vev = ctx.enter_context(tc.tile_pool(name="vev", bufs=3))
                psA = ctx.enter_context(tc.tile_pool(name="psA", bufs=2, space="PSUM"))
                psT = ctx.enter_context(tc.tile_pool(name="psT", bufs=2, space="PSUM"))

                if "A" in phases:
                    kt_all = ktp.tile([128, KC * SK], f32r)
                    vt_all = vtp.tile([128, KC * SK], f32r)
                    # split loads so early k-chunks unblock matmuls sooner
                    for g in range(4):
                        gk = KC // 4
                        nc.sync.dma_start(
                            kt_all[:, g * gk * SK:(g + 1) * gk * SK]
                            .rearrange("p (kc s) -> p kc s", kc=gk),
                            kT[g * gk * 128:(g + 1) * gk * 128, :]
                            .rearrange("(kc p) s -> p kc s", p=128))
                    for g in range(4):
                        gk = KC // 4
                        nc.sync.dma_start(
                            vt_all[:, g * gk * SK:(g + 1) * gk * SK]
                            .rearrange("p (kc s) -> p kc s", kc=gk),
                            vT[g * gk * 128:(g + 1) * gk * 128, :]
                            .rearrange("(kc p) s -> p kc s", p=128))
                for mc in range(MC if "A" in phases else 0):
                    wkc = wkp.tile([128, KC * 128], f32r, tag="wkc")
                    nc.sync.dma_start(
                        wkc[:].rearrange("p (kc m) -> p kc m", kc=KC),
                        wk[:, mc * 128:(mc + 1) * 128]
                        .rearrange("(kc p) m -> p kc m", p=128))
                    ps = psA.tile([128, 1024], f32, tag="psA")
                    for kc in range(KC):
                        lhs = wkc[:, kc * 128:(kc + 1) * 128]
                        for g0, g1, po in ((0, 288, 0), (288, 576, 512)):
                            nc.tensor.matmul(
                                ps[:, po:po + (g1 - g0)], lhs,
                                kt_all[:, kc * SK + g0: kc * SK + g1],
                                start=(kc == 0), stop=(kc == KC - 1))
                    ko = kout.tile([128, SK], f32r, tag="ko")
                    nc.scalar.activation(ko[:, 0:288], ps[:, 0:288], IDENT,
                                         bias=bk_sb[:, mc:mc + 1])
                    nc.scalar.activation(ko[:, 288:576], ps[:, 512:800], IDENT,
                                         bias=bk_sb[:, mc:mc + 1])
                    nc.sync.dma_start(KT_d[mc * 128:(mc + 1) * 128, :], ko[:])

                # ---- V projection (same pools; scheduler interleaves with K) ----
                for mc in range(MC if "A" in phases else 0):
                    wvc = wkp.tile([128, KC * 128], f32r, tag="wkc", name="wvc")
                    nc.sync.dma_start(
                        wvc[:].rearrange("p (kc m) -> p kc m", kc=KC),
                        wv[:, mc * 128:(mc + 1) * 128]
                        .rearrange("(kc p) m -> p kc m", p=128))
                    ps = psA.tile([128, 1024], f32, tag="psA", name="psV")
                    for kc in range(KC):
                        lhs = wvc[:, kc * 128:(kc + 1) * 128]
                        for g0, g1, po in ((0, 288, 0), (288, 576, 512)):
                            nc.tensor.matmul(
                                ps[:, po:po + (g1 - g0)], lhs,
                                vt_all[:, kc * SK + g0: kc * SK + g1],
                                start=(kc == 0), stop=(kc == KC - 1))
                    vo = kout.tile([128, SK], f32r, tag="ko", name="vo")
                    nc.scalar.activation(vo[:, 0:288], ps[:, 0:288], IDENT,
                                         bias=bv_sb[:, mc:mc + 1])
                    nc.scalar.activation(vo[:, 288:576], ps[:, 512:800], IDENT,
                                         bias=bv_sb[:, mc:mc + 1])
                    # transpose VT tile [128, 576] -> V chunks [<=128, 128]
                    for c5 in range(5):
                        sz = 128 if c5 < 4 else 64
                        pt = psT.tile([128, 128], f32r, tag="pt")
                        nc.tensor.transpose(
                            pt[:sz, :], vo[:, c5 * 128: c5 * 128 + sz], ident[:])
                        ve = vev.tile([128, 128], f32r, tag="ve")
                        nc.scalar.copy(ve[:sz, :], pt[:sz, :])
                        nc.sync.dma_start(
                            V_d[c5 * 128: c5 * 128 + sz, mc * 128:(mc + 1) * 128],
                            ve[:sz, :])

            # ---------------- Phase B: QT_d = (q @ wq + bq).T ----------------
            WSTRIP = 1024
            with ExitStack() as ctx:
                qtp = ctx.enter_context(tc.tile_pool(name="qtp", bufs=1))
                wqp = ctx.enter_context(tc.tile_pool(name="wqp", bufs=2))
                qout = ctx.enter_context(tc.tile_pool(name="qout", bufs=3))
                psB = ctx.enter_context(tc.tile_pool(name="psB", bufs=2, space="PSUM"))

                for w in range(SQ // WSTRIP if "B" in phases else 0):
                    qts = qtp.tile([128, KC * WSTRIP], f32r, tag="qts")
                    for g in range(4):
                        gk = KC // 4
                        nc.sync.dma_start(
                            qts[:, g * gk * WSTRIP:(g + 1) * gk * WSTRIP]
                            .rearrange("p (kc s) -> p kc s", kc=gk),
                            qT[g * gk * 128:(g + 1) * gk * 128,
                               w * WSTRIP:(w + 1) * WSTRIP]
                            .rearrange("(kc p) s -> p kc s", p=128))
                    for mc in range(MC):
                        wqc = wqp.tile([128, KC * 128], f32r, tag="wqc")
                        nc.sync.dma_start(
                            wqc[:].rearrange("p (kc m) -> p kc m", kc=KC),
                            wq[:, mc * 128:(mc + 1) * 128]
                            .rearrange("(kc p) m -> p kc m", p=128))
                        ps = psB.tile([128, WSTRIP], f32, tag="psB")
                        for kc in range(KC):
                            lhs = wqc[:, kc * 128:(kc + 1) * 128]
                            for g in range(WSTRIP // 512):
                                nc.tensor.matmul(
                                    ps[:, g * 512:(g + 1) * 512], lhs,
                                    qts[:, kc * WSTRIP + g * 512:
                                        kc * WSTRIP + (g + 1) * 512],
                                    start=(kc == 0), stop=(kc == KC - 1))
                        qo = qout.tile([128, WSTRIP], f32r, tag="qo")
                        nc.scalar.activation(qo[:], ps[:], IDENT,
                                             bias=bq_sb[:, mc:mc + 1])
                        nc.sync.dma_start(
                            QT_d[mc * 128:(mc + 1) * 128,
                                 w * WSTRIP:(w + 1) * WSTRIP], qo[:])

            # ---------------- Phase C: attention per local head ----------------
            # ctxT stays resident across C and D: [128, 16*2048] = 128 KB/part
            ctxp = top.enter_context(tc.tile_pool(name="ctxp", bufs=1))
            ctx_all = ctxp.tile([128, MC * SQ], f32r)

            KEY_CH = [(0, 128), (128, 256), (256, 384), (384, 512), (512, 576)]
            with ExitStack() as ctx:
                khp = ctx.enter_context(tc.tile_pool(name="khp", bufs=2))
                vhp = ctx.enter_context(tc.tile_pool(name="vhp", bufs=2))
                qhp = ctx.enter_context(tc.tile_pool(name="qhp", bufs=3))
                stp = ctx.enter_context(tc.tile_pool(name="stp", bufs=2))
                rbp = ctx.enter_context(tc.tile_pool(name="rbp", bufs=2))
                aop = ctx.enter_context(tc.tile_pool(name="aop", bufs=3))
                rvp = ctx.enter_context(tc.tile_pool(name="rvp", bufs=2))
                psST = ctx.enter_context(tc.tile_pool(name="psST", bufs=2, space="PSUM"))
                psR = ctx.enter_context(tc.tile_pool(name="psR", bufs=2, space="PSUM"))
                psBC = ctx.enter_context(tc.tile_pool(name="psBC", bufs=2, space="PSUM"))
                psCX = ctx.enter_context(tc.tile_pool(name="psCX", bufs=2, space="PSUM"))

                for h in range(HLOC if "C" in phases else 0):
                    kth = khp.tile([128, 2 * SK], f32r, tag="kth")
                    nc.sync.dma_start(
                        kth[:].rearrange("p (c s) -> p c s", c=2),
                        KT_d[h * HD:(h + 1) * HD, :]
                        .rearrange("(c p) s -> p c s", p=128))
                    vha = vhp.tile([128, 4 * HD], f32r, tag="vha")
                    nc.sync.dma_start(
                        vha[:].rearrange("p (c d) -> p c d", c=4),
                        V_d[0:512, h * HD:(h + 1) * HD]
                        .rearrange("(c p) d -> p c d", p=128))
                    vhb = vhp.tile([64, HD], f32r, tag="vhb")
                    nc.sync.dma_start(vhb[:], V_d[512:SK, h * HD:(h + 1) * HD])

                    for rg in range(SQ // 512):
                        qth = qhp.tile([128, 2 * 512], f32r, tag="qth")
                        nc.sync.dma_start(
                            qth[:].rearrange("p (c s) -> p c s", c=2),
                            QT_d[h * HD:(h + 1) * HD, rg * 512:(rg + 1) * 512]
                            .rearrange("(c p) s -> p c s", p=128))

                        stn = stp.tile([128, 5 * 512], f32r, tag="stn")
                        for c5, (k0, k1) in enumerate(KEY_CH):
                            sz = k1 - k0
                            pst = psST.tile([128, 512], f32, tag="pst")
                            for c2 in range(2):
                                nc.tensor.matmul(
                                    pst[:sz, :],
                                    kth[:, c2 * SK + k0: c2 * SK + k1],
                                    qth[:, c2 * 512:(c2 + 1) * 512],
                                    start=(c2 == 0), stop=(c2 == 1))
                            nc.scalar.activation(
                                stn[:sz, c5 * 512:(c5 + 1) * 512], pst[:sz, :],
                                EXP, scale=scale)

                        # column sums over keys via ones-matmul, then 1/x
                        psr = psR.tile([1, 512], f32, tag="psr")
                        for c5, (k0, k1) in enumerate(KEY_CH):
                            sz = k1 - k0
                            nc.tensor.matmul(
                                psr[:, :], ones_col[:sz, :],
                                stn[:sz, c5 * 512:(c5 + 1) * 512],
                                start=(c5 == 0), stop=(c5 == 4))
                        rinv_f = rvp.tile([1, 512], f32, tag="rinv_f")
                        nc.vector.reciprocal(rinv_f[:], psr[:, :])
                        rinv = rvp.tile([1, 512], f32r, tag="rinv")
                        nc.scalar.copy(rinv[:], rinv_f[:])
                        # broadcast rinv across 128 partitions: ones [1,128]^T @ rinv
                        psb = psBC.tile([128, 512], f32, tag="psb")
                        nc.tensor.matmul(psb[:], ones_row[:],
                                         rinv[:],
                                         start=True, stop=True)
                        rb = rbp.tile([128, 512], f32, tag="rb")
                        nc.scalar.copy(rb[:], psb[:])

                        for c5, (k0, k1) in enumerate(KEY_CH):
                            sz = k1 - k0
                            sl = stn[:sz, c5 * 512:(c5 + 1) * 512]
                            aout = aop.tile([128, 512], f32, tag="aout")
                            nc.vector.tensor_mul(aout[:sz, :], sl, rb[:sz, :])
                            nc.sync.dma_start(
                                attn_p[h, k0:k1, rg * 512:(rg + 1) * 512],
                                aout[:sz, :])

                        for c2 in range(2):
                            pcx = psCX.tile([128, 512], f32, tag="pcx")
                            for c5, (k0, k1) in enumerate(KEY_CH):
                                sz = k1 - k0
                                if c5 < 4:
                                    vsl = vha[:, c5 * HD + c2 * 128:
                                              c5 * HD + (c2 + 1) * 128]
                                else:
                                    vsl = vhb[:, c2 * 128:(c2 + 1) * 128]
                                nc.tensor.matmul(
                                    pcx[:], vsl,
                                    stn[:sz, c5 * 512:(c5 + 1) * 512],
                                    start=(c5 == 0), stop=(c5 == 4))
                            nc.vector.tensor_mul(
                                ctx_all[:, (h * 2 + c2) * SQ + rg * 512:
                                        (h * 2 + c2) * SQ + (rg + 1) * 512],
                                pcx[:], rb[:])

            # ---------------- Phase D: outT_p = wo.T @ ctxT + bo/2 ----------------
            with ExitStack() as ctx:
                wop = ctx.enter_context(tc.tile_pool(name="wop", bufs=2))
                opool = ctx.enter_context(tc.tile_pool(name="opool", bufs=4))
                psD = ctx.enter_context(tc.tile_pool(name="psD", bufs=2, space="PSUM"))

                for oc in range(D // 128 if "D" in phases else 0):
                    woc = wop.tile([128, MC * 128], f32r, tag="woc")
                    nc.sync.dma_start(
                        woc[:].rearrange("p (fc m) -> p fc m", fc=MC),
                        wo[:, oc * 128:(oc + 1) * 128]
                        .rearrange("(fc p) m -> p fc m", p=128))
                    pss = [psD.tile([128, 512], f32, tag=f"psD{rg}",
                                    name=f"psD{rg}") for rg in range(4)]
                    for fc in range(MC):
                        lhs = woc[:, fc * 128:(fc + 1) * 128]
                        for rg in range(4):
                            nc.tensor.matmul(
                                pss[rg][:], lhs,
                                ctx_all[:, fc * SQ + rg * 512:
                                        fc * SQ + (rg + 1) * 512],
                                start=(fc == 0), stop=(fc == MC - 1))
                    for rg in range(4):
                        osb = opool.tile([128, 512], f32, tag="osb")
                        nc.scalar.activation(osb[:], pss[rg][:], IDENT,
                                             bias=bo2_sb[:, oc:oc + 1])
                        nc.sync.dma_start(
                            outT_p[oc * 128:(oc + 1) * 128,
                                   rg * 512:(rg + 1) * 512], osb[:])

    nc.compile()
    _BUILD_CACHE[key] = nc
    return nc


def _make_in_maps(query, key, value, Wq, bq, Wk, bk, Wv, bv, Wo, bo):
    f = np.float32
    bo2 = (np.asarray(bo, f) / 2.0).astype(f)
    in_maps = []
    for c in range(N_CORES):
        b, s = c // 2, c % 2
        sl = slice(s * DLOC, (s + 1) * DLOC)
        in_maps.append({
            "qT": np.ascontiguousarray(np.asarray(query[b], f).T),
            "kT": np.ascontiguousarray(np.asarray(key[b], f).T),
            "vT": np.ascontiguousarray(np.asarray(value[b], f).T),
            "wq": np.ascontiguousarray(np.asarray(Wq, f)[:, sl]),
            "wk": np.ascontiguousarray(np.asarray(Wk, f)[:, sl]),
            "wv": np.ascontiguousarray(np.asarray(Wv, f)[:, sl]),
            "wo": np.ascontiguousarray(np.asarray(Wo, f)[sl, :]),
            "bq": np.ascontiguousarray(np.asarray(bq, f)[sl]),
            "bk": np.ascontiguousarray(np.asarray(bk, f)[sl]),
            "bv": np.ascontiguousarray(np.asarray(bv, f)[sl]),
            "bo2": bo2,
        })
    return in_maps


def _gather(results):
    out = np.empty((B, SQ, D), np.float32)
    attn = np.empty((B, SQ, SK), np.float32)
    for b in range(B):
        r0, r1 = results[2 * b], results[2 * b + 1]
        out[b] = (r0["outT_p"] + r1["outT_p"]).T
        attn[b] = ((r0["attn_p"].sum(axis=0) + r1["attn_p"].sum(axis=0)) / H).T
    return out, attn


def kernel(query, key, value, Wq, bq, Wk, bk, Wv, bv, Wo, bo, temperature):
    from concourse.bass_utils import run_bass_kernel_spmd

    temp = float(np.asarray(temperature))
    nc = _build(SCALE / temp)
    in_maps = _make_in_maps(query, key, value, Wq, bq, Wk, bk, Wv, bv, Wo, bo)
    res = run_bass_kernel_spmd(nc, in_maps, core_ids=list(range(N_CORES)))
    return _gather(res.results)


# revision 14
# speedup vs baseline: 1.2607x; 1.1181x over previous
"""CrossModalAttention Trainium2 kernel.

Full-input contract: kernel(**inputs) takes the unsharded inputs from
setup_inputs() and returns (out [4,2048,4096], attn_mean [4,2048,576]) as fp32.

Sharding (8 NeuronCores): 4-way data parallel over batch x 2-way tensor
parallel over heads (Megatron): column-parallel Q/K/V projections (each shard
owns 8 of 16 heads = 2048 of 4096 projected features), row-parallel out_proj
(host sums the two partial outputs per batch), and the head-mean attention
weights are summed across the two head shards on the host.

Device kernel (identical SPMD program, per-core data differs):
  inputs (per core, fp32):
    qT [4096,2048]  = query[b].T          wq/wk/wv [4096,2048] column slices
    kT [4096, 576]  = key[b].T            wo [2048,4096] row slice
    vT [4096, 576]  = value[b].T          bq/bk/bv [2048] slices, bo2 [4096]=bo/2
  outputs:
    outT_p [4096,2048]  = (ctx @ wo + bo/2).T   (partial over head shards)
    attn_p [8,576,2048] = per-local-head normalized attention (S^T layout)

All matmuls run as float32r (full PE rate for free-dim >= 256, fp32 storage).
Activations flow "feature-major" (transposed) so the contraction dim is always
on SBUF partitions; softmax runs on S^T tiles with column sums computed by
ones-vector matmuls on the PE and broadcast back via a rank-1 matmul.
"""

import numpy as np

N_CORES = 8
B = 4
SQ = 2048
SK = 576
D = 4096
H = 16
HD = 256
HLOC = 8          # heads per shard
DLOC = 2048       # projected features per shard
KC = D // 128     # 32 contraction chunks
SCALE = HD ** -0.5

_BUILD_CACHE = {}


def _build(scale: float, phases: str = "ABCD"):
    """Build + compile the per-core Bass program. scale = SCALE / temperature."""
    key = (scale, phases)
    if key in _BUILD_CACHE:
        return _BUILD_CACHE[key]

    from contextlib import ExitStack

    import concourse.bacc as bacc
    import concourse.tile as tile
    from concourse import mybir
    from concourse.masks import make_identity

    f32 = mybir.dt.float32
    f32r = mybir.dt.float32r
    IDENT = mybir.ActivationFunctionType.Identity
    EXP = mybir.ActivationFunctionType.Exp

    nc = bacc.Bacc("TRN2", target_bir_lowering=False, debug=False,
                   enable_asserts=True, num_devices=1)

    qT = nc.dram_tensor("qT", [D, SQ], f32r, kind="ExternalInput").ap()
    kT = nc.dram_tensor("kT", [D, SK], f32r, kind="ExternalInput").ap()
    vT = nc.dram_tensor("vT", [D, SK], f32r, kind="ExternalInput").ap()
    wq = nc.dram_tensor("wq", [D, DLOC], f32r, kind="ExternalInput").ap()
    wk = nc.dram_tensor("wk", [D, DLOC], f32r, kind="ExternalInput").ap()
    wv = nc.dram_tensor("wv", [D, DLOC], f32r, kind="ExternalInput").ap()
    wo = nc.dram_tensor("wo", [DLOC, D], f32r, kind="ExternalInput").ap()
    bq = nc.dram_tensor("bq", [DLOC], f32, kind="ExternalInput").ap()
    bk = nc.dram_tensor("bk", [DLOC], f32, kind="ExternalInput").ap()
    bv = nc.dram_tensor("bv", [DLOC], f32, kind="ExternalInput").ap()
    bo2 = nc.dram_tensor("bo2", [D], f32, kind="ExternalInput").ap()

    outT_p = nc.dram_tensor("outT_p", [D, SQ], f32, kind="ExternalOutput").ap()
    attn_p = nc.dram_tensor("attn_p", [HLOC, SK, SQ], f32, kind="ExternalOutput").ap()

    # DRAM scratch for inter-phase spills
    KT_d = nc.dram_tensor("KT_d", [DLOC, SK], f32r, kind="Internal").ap()
    V_d = nc.dram_tensor("V_d", [SK, DLOC], f32r, kind="Internal").ap()
    QT_d = nc.dram_tensor("QT_d", [DLOC, SQ], f32r, kind="Internal").ap()

    MC = DLOC // 128  # 16 output-feature chunks per shard

    with tile.TileContext(nc) as tc:
        with ExitStack() as top:
            const = top.enter_context(tc.tile_pool(name="const", bufs=1))
            ident_f = const.tile([128, 128], f32)
            make_identity(nc, ident_f)
            ident = const.tile([128, 128], f32r)
            nc.scalar.copy(ident[:], ident_f[:])
            ones_f = const.tile([128, 1], f32)
            nc.vector.memset(ones_f[:], 1.0)
            ones_col = const.tile([128, 1], f32r)
            nc.scalar.copy(ones_col[:], ones_f[:])
            ones_rf = const.tile([1, 128], f32)
            nc.vector.memset(ones_rf[:], 1.0)
            ones_row = const.tile([1, 128], f32r)
            nc.scalar.copy(ones_row[:], ones_rf[:])
            bq_sb = const.tile([128, MC], f32)
            nc.sync.dma_start(bq_sb[:], bq.rearrange("(m p) -> p m", p=128))
            bk_sb = const.tile([128, MC], f32)
            nc.sync.dma_start(bk_sb[:], bk.rearrange("(m p) -> p m", p=128))
            bv_sb = const.tile([128, MC], f32)
            nc.sync.dma_start(bv_sb[:], bv.rearrange("(m p) -> p m", p=128))
            bo2_sb = const.tile([128, KC], f32)
            nc.sync.dma_start(bo2_sb[:], bo2.rearrange("(m p) -> p m", p=128))

            # ------- Phase A: KT_d = (k@wk+bk).T and V_d = v@wv+bv (interleaved) -------
            with ExitStack() as ctx:
                ktp = ctx.enter_context(tc.tile_pool(name="ktp", bufs=1))
                vtp = ctx.enter_context(tc.tile_pool(name="vtp", bufs=1))
                wkp = ctx.enter_context(tc.tile_pool(name="wkp", bufs=2))
                kout = ctx.enter_context(tc.tile_pool(name="kout", bufs=3))
                vev = ctx.enter_context(tc.tile_pool(name="vev", bufs=3))
                psA = ctx.enter_context(tc.tile_pool(name="psA", bufs=3, space="PSUM"))
                psT = ctx.enter_context(tc.tile_pool(name="psT", bufs=2, space="PSUM"))

                if "A" in phases:
                    kt_all = ktp.tile([128, KC * SK], f32r)
                    vt_all = vtp.tile([128, KC * SK], f32r)
                    # split loads so early k-chunks unblock matmuls sooner
                    for g in range(4):
                        gk = KC // 4
                        nc.sync.dma_start(
                            kt_all[:, g * gk * SK:(g + 1) * gk * SK]
                            .rearrange("p (kc s) -> p kc s", kc=gk),
                            kT[g * gk * 128:(g + 1) * gk * 128, :]
                            .rearrange("(kc p) s -> p kc s", p=128))
                    for g in range(4):
                        gk = KC // 4
                        nc.sync.dma_start(
                            vt_all[:, g * gk * SK:(g + 1) * gk * SK]
                            .rearrange("p (kc s) -> p kc s", kc=gk),
                            vT[g * gk * 128:(g + 1) * gk * 128, :]
                            .rearrange("(kc p) s -> p kc s", p=128))
                for mc in range(MC if "A" in phases else 0):
                    wkc = wkp.tile([128, KC * 128], f32r, tag="wkc")
                    nc.sync.dma_start(
                        wkc[:].rearrange("p (kc m) -> p kc m", kc=KC),
                        wk[:, mc * 128:(mc + 1) * 128]
                        .rearrange("(kc p) m -> p kc m", p=128))
                    ps = psA.tile([128, 1024], f32, tag="psA")
                    for kc in range(KC):
                        lhs = wkc[:, kc * 128:(kc + 1) * 128]
                        for g0, g1, po in ((0, 288, 0), (288, 576, 512)):
                            nc.tensor.matmul(
                                ps[:, po:po + (g1 - g0)], lhs,
                                kt_all[:, kc * SK + g0: kc * SK + g1],
                                start=(kc == 0), stop=(kc == KC - 1))
                    ko = kout.tile([128, SK], f32r, tag="ko")
                    nc.scalar.activation(ko[:, 0:288], ps[:, 0:288], IDENT,
                                         bias=bk_sb[:, mc:mc + 1])
                    nc.scalar.activation(ko[:, 288:576], ps[:, 512:800], IDENT,
                                         bias=bk_sb[:, mc:mc + 1])
                    nc.sync.dma_start(KT_d[mc * 128:(mc + 1) * 128, :], ko[:])

                # ---- V projection (same pools; scheduler interleaves with K) ----
                for mc in range(MC if "A" in phases else 0):
                    wvc = wkp.tile([128, KC * 128], f32r, tag="wkc", name="wvc")
                    nc.sync.dma_start(
                        wvc[:].rearrange("p (kc m) -> p kc m", kc=KC),
                        wv[:, mc * 128:(mc + 1) * 128]
                        .rearrange("(kc p) m -> p kc m", p=128))
                    ps = psA.tile([128, 1024], f32, tag="psA", name="psV")
                    for kc in range(KC):
                        lhs = wvc[:, kc * 128:(kc + 1) * 128]
                        for g0, g1, po in ((0, 288, 0), (288, 576, 512)):
                            nc.tensor.matmul(
                                ps[:, po:po + (g1 - g0)], lhs,
                                vt_all[:, kc * SK + g0: kc * SK + g1],
                                start=(kc == 0), stop=(kc == KC - 1))
                    vo = kout.tile([128, SK], f32r, tag="ko", name="vo")
                    nc.scalar.activation(vo[:, 0:288], ps[:, 0:288], IDENT,
                                         bias=bv_sb[:, mc:mc + 1])
                    nc.scalar.activation(vo[:, 288:576], ps[:, 512:800], IDENT,
                                         bias=bv_sb[:, mc:mc + 1])
                    # transpose VT tile [128, 576] -> V chunks [<=128, 128]
                    for c5 in range(5):
                        sz = 128 if c5 < 4 else 64
                        pt = psT.tile([128, 128], f32r, tag="pt")
                        nc.tensor.transpose(
                            pt[:sz, :], vo[:, c5 * 128: c5 * 128 + sz], ident[:])
                        ve = vev.tile([128, 128], f32r, tag="ve")
                        nc.scalar.copy(ve[:sz, :], pt[:sz, :])
                        nc.sync.dma_start(
                            V_d[c5 * 128: c5 * 128 + sz, mc * 128:(mc + 1) * 128],
                            ve[:sz, :])

            # ---------------- Phase B: QT_d = (q @ wq + bq).T ----------------
            WSTRIP = 1024
            with ExitStack() as ctx:
                qtp = ctx.enter_context(tc.tile_pool(name="qtp", bufs=1))
                wqp = ctx.enter_context(tc.tile_pool(name="wqp", bufs=2))
                qout = ctx.enter_context(tc.tile_pool(name="qout", bufs=3))
                psB = ctx.enter_context(tc.tile_pool(name="psB", bufs=3, space="PSUM"))

                for w in range(SQ // WSTRIP if "B" in phases else 0):
                    qts = qtp.tile([128, KC * WSTRIP], f32r, tag="qts")
                    for g in range(4):
                        gk = KC // 4
                        nc.sync.dma_start(
                            qts[:, g * gk * WSTRIP:(g + 1) * gk * WSTRIP]
                            .rearrange("p (kc s) -> p kc s", kc=gk),
                            qT[g * gk * 128:(g + 1) * gk * 128,
                               w * WSTRIP:(w + 1) * WSTRIP]
                            .rearrange("(kc p) s -> p kc s", p=128))
                    for mc in range(MC):
                        wqc = wqp.tile([128, KC * 128], f32r, tag="wqc")
                        nc.sync.dma_start(
                            wqc[:].rearrange("p (kc m) -> p kc m", kc=KC),
                            wq[:, mc * 128:(mc + 1) * 128]
                            .rearrange("(kc p) m -> p kc m", p=128))
                        ps = psB.tile([128, WSTRIP], f32, tag="psB")
                        for kc in range(KC):
                            lhs = wqc[:, kc * 128:(kc + 1) * 128]
                            for g in range(WSTRIP // 512):
                                nc.tensor.matmul(
                                    ps[:, g * 512:(g + 1) * 512], lhs,
                                    qts[:, kc * WSTRIP + g * 512:
                                        kc * WSTRIP + (g + 1) * 512],
                                    start=(kc == 0), stop=(kc == KC - 1))
                        qo = qout.tile([128, WSTRIP], f32r, tag="qo")
                        nc.scalar.activation(qo[:], ps[:], IDENT,
                                             bias=bq_sb[:, mc:mc + 1])
                        nc.sync.dma_start(
                            QT_d[mc * 128:(mc + 1) * 128,
                                 w * WSTRIP:(w + 1) * WSTRIP], qo[:])

            # ---------------- Phase C: attention per local head ----------------
            # ctxT stays resident across C and D: [128, 16*2048] = 128 KB/part
            ctxp = top.enter_context(tc.tile_pool(name="ctxp", bufs=1))
            ctx_all = ctxp.tile([128, MC * SQ], f32r)

            KEY_CH = [(0, 128), (128, 256), (256, 384), (384, 512), (512, 576)]
            with ExitStack() as ctx:
                khp = ctx.enter_context(tc.tile_pool(name="khp", bufs=2))
                vhp = ctx.enter_context(tc.tile_pool(name="vhp", bufs=2))
                qhp = ctx.enter_context(tc.tile_pool(name="qhp", bufs=3))
                stp = ctx.enter_context(tc.tile_pool(name="stp", bufs=2))
                rbp = ctx.enter_context(tc.tile_pool(name="rbp", bufs=2))
                aop = ctx.enter_context(tc.tile_pool(name="aop", bufs=3))
                rvp = ctx.enter_context(tc.tile_pool(name="rvp", bufs=2))
                psST = ctx.enter_context(tc.tile_pool(name="psST", bufs=2, space="PSUM"))
                psR = ctx.enter_context(tc.tile_pool(name="psR", bufs=2, space="PSUM"))
                psBC = ctx.enter_context(tc.tile_pool(name="psBC", bufs=2, space="PSUM"))
                psCX = ctx.enter_context(tc.tile_pool(name="psCX", bufs=2, space="PSUM"))

                for h in range(HLOC if "C" in phases else 0):
                    kth = khp.tile([128, 2 * SK], f32r, tag="kth")
                    nc.sync.dma_start(
                        kth[:].rearrange("p (c s) -> p c s", c=2),
                        KT_d[h * HD:(h + 1) * HD, :]
                        .rearrange("(c p) s -> p c s", p=128))
                    vha = vhp.tile([128, 4 * HD], f32r, tag="vha")
                    nc.sync.dma_start(
                        vha[:].rearrange("p (c d) -> p c d", c=4),
                        V_d[0:512, h * HD:(h + 1) * HD]
                        .rearrange("(c p) d -> p c d", p=128))
                    vhb = vhp.tile([64, HD], f32r, tag="vhb")
                    nc.sync.dma_start(vhb[:], V_d[512:SK, h * HD:(h + 1) * HD])

                    for rg in range(SQ // 512):
                        qth = qhp.tile([128, 2 * 512], f32r, tag="qth")
                        nc.sync.dma_start(
                            qth[:].rearrange("p (c s) -> p c s", c=2),
                            QT_d[h * HD:(h + 1) * HD, rg * 512:(rg + 1) * 512]
                            .rearrange("(c p) s -> p c s", p=128))

                        stn = stp.tile([128, 5 * 512], f32r, tag="stn")
                        for c5, (k0, k1) in enumerate(KEY_CH):
                            sz = k1 - k0
                            pst = psST.tile([128, 512], f32, tag="pst")
                            for c2 in range(2):
                                nc.tensor.matmul(
                                    pst[:sz, :],
                                    kth[:, c2 * SK + k0: c2 * SK + k1],
                                    qth[:, c2 * 512:(c2 + 1) * 512],
                                    start=(c2 == 0), stop=(c2 == 1))
                            nc.scalar.activation(
                                stn[:sz, c5 * 512:(c5 + 1) * 512], pst[:sz, :],
                                EXP, scale=scale)

                        # column sums over keys via ones-matmul, then 1/x
                        psr = psR.tile([1, 512], f32, tag="psr")
                        for c5, (k0, k1) in enumerate(KEY_CH):
                            sz = k1 - k0
                            nc.tensor.matmul(
                                psr[:, :], ones_col[:sz, :],
                                stn[:sz, c5 * 512:(c5 + 1) * 512],
                                start=(c5 == 0), stop=(c5 == 4))
                        rinv_f = rvp.tile([1, 512], f32, tag="rinv_f")
                        nc.vector.reciprocal(rinv_f[:], psr[:, :])
                        rinv = rvp.tile([1, 512], f32r, tag="rinv")
                        nc.scalar.copy(rinv[:], rinv_f[:])
                        # broadcast rinv across 128 partitions: ones [1,128]^T @ rinv
                        psb = psBC.tile([128, 512], f32, tag="psb")
                        nc.tensor.matmul(psb[:], ones_row[:],
                                         rinv[:],
                                         start=True, stop=True)
                        rb = rbp.tile([128, 512], f32, tag="rb")
                        nc.scalar.copy(rb[:], psb[:])

                        for c5, (k0, k1) in enumerate(KEY_CH):
                            sz = k1 - k0
                            sl = stn[:sz, c5 * 512:(c5 + 1) * 512]
                            aout = aop.tile([128, 512], f32, tag="aout")
                            nc.vector.tensor_mul(aout[:sz, :], sl, rb[:sz, :])
                            nc.sync.dma_start(
                                attn_p[h, k0:k1, rg * 512:(rg + 1) * 512],
                                aout[:sz, :])

                        for c2 in range(2):
                            pcx = psCX.tile([128, 512], f32, tag="pcx")
                            for c5, (k0, k1) in enumerate(KEY_CH):
                                sz = k1 - k0
                                if c5 < 4:
                                    vsl = vha[:, c5 * HD + c2 * 128:
                                              c5 * HD + (c2 + 1) * 128]
                                else:
                                    vsl = vhb[:, c2 * 128:(c2 + 1) * 128]
                                nc.tensor.matmul(
                                    pcx[:], vsl,
                                    stn[:sz, c5 * 512:(c5 + 1) * 512],
                                    start=(c5 == 0), stop=(c5 == 4))
                            nc.vector.tensor_mul(
                                ctx_all[:, (h * 2 + c2) * SQ + rg * 512:
                                        (h * 2 + c2) * SQ + (rg + 1) * 512],
                                pcx[:], rb[:])

            # ---------------- Phase D: outT_p = wo.T @ ctxT + bo/2 ----------------
            with ExitStack() as ctx:
                wop = ctx.enter_context(tc.tile_pool(name="wop", bufs=3))
                opool = ctx.enter_context(tc.tile_pool(name="opool", bufs=4))
                psD = ctx.enter_context(tc.tile_pool(name="psD", bufs=2, space="PSUM"))

                for oc in range(D // 128 if "D" in phases else 0):
                    woc = wop.tile([128, MC * 128], f32r, tag="woc")
                    nc.sync.dma_start(
                        woc[:].rearrange("p (fc m) -> p fc m", fc=MC),
                        wo[:, oc * 128:(oc + 1) * 128]
                        .rearrange("(fc p) m -> p fc m", p=128))
                    pss = [psD.tile([128, 512], f32, tag=f"psD{rg}",
                                    name=f"psD{rg}") for rg in range(4)]
                    for fc in range(MC):
                        lhs = woc[:, fc * 128:(fc + 1) * 128]
                        for rg in range(4):
                            nc.tensor.matmul(
                                pss[rg][:], lhs,
                                ctx_all[:, fc * SQ + rg * 512:
                                        fc * SQ + (rg + 1) * 512],
                                start=(fc == 0), stop=(fc == MC - 1))
                    for rg in range(4):
                        osb = opool.tile([128, 512], f32, tag="osb")
                        nc.scalar.activation(osb[:], pss[rg][:], IDENT,
                                             bias=bo2_sb[:, oc:oc + 1])
                        nc.sync.dma_start(
                            outT_p[oc * 128:(oc + 1) * 128,
                                   rg * 512:(rg + 1) * 512], osb[:])

    nc.compile()
    _BUILD_CACHE[key] = nc
    return nc


def _make_in_maps(query, key, value, Wq, bq, Wk, bk, Wv, bv, Wo, bo):
    f = np.float32
    bo2 = (np.asarray(bo, f) / 2.0).astype(f)
    in_maps = []
    for c in range(N_CORES):
        b, s = c // 2, c % 2
        sl = slice(s * DLOC, (s + 1) * DLOC)
        in_maps.append({
            "qT": np.ascontiguousarray(np.asarray(query[b], f).T),
            "kT": np.ascontiguousarray(np.asarray(key[b], f).T),
            "vT": np.ascontiguousarray(np.asarray(value[b], f).T),
            "wq": np.ascontiguousarray(np.asarray(Wq, f)[:, sl]),
            "wk": np.ascontiguousarray(np.asarray(Wk, f)[:, sl]),
            "wv": np.ascontiguousarray(np.asarray(Wv, f)[:, sl]),
            "wo": np.ascontiguousarray(np.asarray(Wo, f)[sl, :]),
            "bq": np.ascontiguousarray(np.asarray(bq, f)[sl]),
            "bk": np.ascontiguousarray(np.asarray(bk, f)[sl]),
            "bv": np.ascontiguousarray(np.asarray(bv, f)[sl]),
            "bo2": bo2,
        })
    return in_maps


def _gather(results):
    out = np.empty((B, SQ, D), np.float32)
    attn = np.empty((B, SQ, SK), np.float32)
    for b in range(B):
        r0, r1 = results[2 * b], results[2 * b + 1]
        out[b] = (r0["outT_p"] + r1["outT_p"]).T
        attn[b] = ((r0["attn_p"].sum(axis=0) + r1["attn_p"].sum(axis=0)) / H).T
    return out, attn


def _enable_jax_compile_cache():
    try:
        import jax
        jax.config.update("jax_compilation_cache_dir",
                          "/tmp/jax_neff_cache_cma74002286510558")
        jax.config.update("jax_persistent_cache_min_compile_time_secs", 0.0)
        jax.config.update("jax_persistent_cache_min_entry_size_bytes", -1)
    except Exception:
        pass


def kernel(query, key, value, Wq, bq, Wk, bk, Wv, bv, Wo, bo, temperature):
    from concourse.bass_utils import run_bass_kernel_spmd

    _enable_jax_compile_cache()

    temp = float(np.asarray(temperature))
    nc = _build(SCALE / temp)
    in_maps = _make_in_maps(query, key, value, Wq, bq, Wk, bk, Wv, bv, Wo, bo)
    res = run_bass_kernel_spmd(nc, in_maps, core_ids=list(range(N_CORES)))
    return _gather(res.results)


# revision 16
# speedup vs baseline: 2.2356x; 1.7733x over previous
"""CrossModalAttention Trainium2 kernel.

Full-input contract: kernel(**inputs) takes the unsharded inputs from
setup_inputs() and returns (out [4,2048,4096], attn_mean [4,2048,576]) as fp32.

Sharding (8 NeuronCores): 4-way data parallel over batch x 2-way tensor
parallel over heads (Megatron): column-parallel Q/K/V projections (each shard
owns 8 of 16 heads = 2048 of 4096 projected features), row-parallel out_proj
(host sums the two partial outputs per batch), and the head-mean attention
weights are summed across the two head shards on the host.

Device kernel (identical SPMD program, per-core data differs):
  inputs (per core, fp32):
    qT [4096,2048]  = query[b].T          wq/wk/wv [4096,2048] column slices
    kT [4096, 576]  = key[b].T            wo [2048,4096] row slice
    vT [4096, 576]  = value[b].T          bq/bk/bv [2048] slices, bo2 [4096]=bo/2
  outputs:
    outT_p [4096,2048]  = (ctx @ wo + bo/2).T   (partial over head shards)
    attn_p [8,576,2048] = per-local-head normalized attention (S^T layout)

All matmuls run as float32r (full PE rate for free-dim >= 256, fp32 storage).
Activations flow "feature-major" (transposed) so the contraction dim is always
on SBUF partitions; softmax runs on S^T tiles with column sums computed by
ones-vector matmuls on the PE and broadcast back via a rank-1 matmul.
"""

import numpy as np

N_CORES = 8
B = 4
SQ = 2048
SK = 576
D = 4096
H = 16
HD = 256
HLOC = 8          # heads per shard
DLOC = 2048       # projected features per shard
KC = D // 128     # 32 contraction chunks
SCALE = HD ** -0.5

_BUILD_CACHE = {}


def _build(scale: float, phases: str = "ABCD"):
    """Build + compile the per-core Bass program. scale = SCALE / temperature."""
    key = (scale, phases)
    if key in _BUILD_CACHE:
        return _BUILD_CACHE[key]

    from contextlib import ExitStack

    import concourse.bacc as bacc
    import concourse.tile as tile
    from concourse import mybir
    from concourse.masks import make_identity

    f32 = mybir.dt.float32
    f32r = mybir.dt.float32r
    IDENT = mybir.ActivationFunctionType.Identity
    EXP = mybir.ActivationFunctionType.Exp

    nc = bacc.Bacc("TRN2", target_bir_lowering=False, debug=False,
                   enable_asserts=True, num_devices=1)

    qT = nc.dram_tensor("qT", [D, SQ], f32r, kind="ExternalInput").ap()
    kT = nc.dram_tensor("kT", [D, SK], f32r, kind="ExternalInput").ap()
    vT = nc.dram_tensor("vT", [D, SK], f32r, kind="ExternalInput").ap()
    wq = nc.dram_tensor("wq", [D, DLOC], f32r, kind="ExternalInput").ap()
    wk = nc.dram_tensor("wk", [D, DLOC], f32r, kind="ExternalInput").ap()
    wv = nc.dram_tensor("wv", [D, DLOC], f32r, kind="ExternalInput").ap()
    wo = nc.dram_tensor("wo", [DLOC, D], f32r, kind="ExternalInput").ap()
    bq = nc.dram_tensor("bq", [DLOC], f32, kind="ExternalInput").ap()
    bk = nc.dram_tensor("bk", [DLOC], f32, kind="ExternalInput").ap()
    bv = nc.dram_tensor("bv", [DLOC], f32, kind="ExternalInput").ap()
    bo2 = nc.dram_tensor("bo2", [D], f32, kind="ExternalInput").ap()

    outT_p = nc.dram_tensor("outT_p", [D, SQ], f32, kind="ExternalOutput").ap()
    attn_p = nc.dram_tensor("attn_p", [HLOC, SK, SQ], f32, kind="ExternalOutput").ap()

    # DRAM scratch for inter-phase spills
    KT_d = nc.dram_tensor("KT_d", [DLOC, SK], f32r, kind="Internal").ap()
    V_d = nc.dram_tensor("V_d", [SK, DLOC], f32r, kind="Internal").ap()
    QT_d = nc.dram_tensor("QT_d", [DLOC, SQ], f32r, kind="Internal").ap()

    MC = DLOC // 128  # 16 output-feature chunks per shard

    with tile.TileContext(nc) as tc:
        with ExitStack() as top:
            const = top.enter_context(tc.tile_pool(name="const", bufs=1))
            ident_f = const.tile([128, 128], f32)
            make_identity(nc, ident_f)
            ident = const.tile([128, 128], f32r)
            nc.scalar.copy(ident[:], ident_f[:])
            ones_f = const.tile([128, 1], f32)
            nc.vector.memset(ones_f[:], 1.0)
            ones_col = const.tile([128, 1], f32r)
            nc.scalar.copy(ones_col[:], ones_f[:])
            ones_rf = const.tile([1, 128], f32)
            nc.vector.memset(ones_rf[:], 1.0)
            ones_row = const.tile([1, 128], f32r)
            nc.scalar.copy(ones_row[:], ones_rf[:])
            bq_sb = const.tile([128, MC], f32)
            nc.sync.dma_start(bq_sb[:], bq.rearrange("(m p) -> p m", p=128))
            bk_sb = const.tile([128, MC], f32)
            nc.sync.dma_start(bk_sb[:], bk.rearrange("(m p) -> p m", p=128))
            bv_sb = const.tile([128, MC], f32)
            nc.sync.dma_start(bv_sb[:], bv.rearrange("(m p) -> p m", p=128))
            bo2_sb = const.tile([128, KC], f32)
            nc.sync.dma_start(bo2_sb[:], bo2.rearrange("(m p) -> p m", p=128))

            # ------- Phase A: KT_d = (k@wk+bk).T and V_d = v@wv+bv (interleaved) -------
            with ExitStack() as ctx:
                ktp = ctx.enter_context(tc.tile_pool(name="ktp", bufs=1))
                vtp = ctx.enter_context(tc.tile_pool(name="vtp", bufs=1))
                wkp = ctx.enter_context(tc.tile_pool(name="wkp", bufs=2))
                kout = ctx.enter_context(tc.tile_pool(name="kout", bufs=3))
                vev = ctx.enter_context(tc.tile_pool(name="vev", bufs=2))
                psA = ctx.enter_context(tc.tile_pool(name="psA", bufs=3, space="PSUM"))
                psT = ctx.enter_context(tc.tile_pool(name="psT", bufs=2, space="PSUM"))

                if "A" in phases:
                    kt_all = ktp.tile([128, KC * SK], f32r)
                    vt_all = vtp.tile([128, KC * SK], f32r)
                    # split loads so early k-chunks unblock matmuls sooner
                    for g in range(4):
                        gk = KC // 4
                        nc.sync.dma_start(
                            kt_all[:, g * gk * SK:(g + 1) * gk * SK]
                            .rearrange("p (kc s) -> p kc s", kc=gk),
                            kT[g * gk * 128:(g + 1) * gk * 128, :]
                            .rearrange("(kc p) s -> p kc s", p=128))
                    for g in range(4):
                        gk = KC // 4
                        nc.sync.dma_start(
                            vt_all[:, g * gk * SK:(g + 1) * gk * SK]
                            .rearrange("p (kc s) -> p kc s", kc=gk),
                            vT[g * gk * 128:(g + 1) * gk * 128, :]
                            .rearrange("(kc p) s -> p kc s", p=128))
                for mc in range(MC if "A" in phases else 0):
                    wkc = wkp.tile([128, KC * 128], f32r, tag="wkc")
                    nc.sync.dma_start(
                        wkc[:].rearrange("p (kc m) -> p kc m", kc=KC),
                        wk[:, mc * 128:(mc + 1) * 128]
                        .rearrange("(kc p) m -> p kc m", p=128))
                    ps = psA.tile([128, 1024], f32, tag="psA")
                    for kc in range(KC):
                        lhs = wkc[:, kc * 128:(kc + 1) * 128]
                        for g0, g1, po in ((0, 288, 0), (288, 576, 512)):
                            nc.tensor.matmul(
                                ps[:, po:po + (g1 - g0)], lhs,
                                kt_all[:, kc * SK + g0: kc * SK + g1],
                                start=(kc == 0), stop=(kc == KC - 1))
                    ko = kout.tile([128, SK], f32r, tag="ko")
                    nc.scalar.activation(ko[:, 0:288], ps[:, 0:288], IDENT,
                                         bias=bk_sb[:, mc:mc + 1])
                    nc.scalar.activation(ko[:, 288:576], ps[:, 512:800], IDENT,
                                         bias=bk_sb[:, mc:mc + 1])
                    nc.sync.dma_start(KT_d[mc * 128:(mc + 1) * 128, :], ko[:])

                # ---- V projection (same pools; scheduler interleaves with K) ----
                for mcp in range(MC // 2 if "A" in phases else 0):
                    ves = []
                    for half in range(2):
                        mc = mcp * 2 + half
                        wvc = wkp.tile([128, KC * 128], f32r, tag="wkc",
                                       name="wvc")
                        nc.sync.dma_start(
                            wvc[:].rearrange("p (kc m) -> p kc m", kc=KC),
                            wv[:, mc * 128:(mc + 1) * 128]
                            .rearrange("(kc p) m -> p kc m", p=128))
                        ps = psA.tile([128, 1024], f32, tag="psA", name="psV")
                        for kc in range(KC):
                            lhs = wvc[:, kc * 128:(kc + 1) * 128]
                            for g0, g1, po in ((0, 288, 0), (288, 576, 512)):
                                nc.tensor.matmul(
                                    ps[:, po:po + (g1 - g0)], lhs,
                                    vt_all[:, kc * SK + g0: kc * SK + g1],
                                    start=(kc == 0), stop=(kc == KC - 1))
                        vo = kout.tile([128, SK], f32r, tag="ko", name="vo")
                        nc.scalar.activation(vo[:, 0:288], ps[:, 0:288], IDENT,
                                             bias=bv_sb[:, mc:mc + 1])
                        nc.scalar.activation(vo[:, 288:576], ps[:, 512:800],
                                             IDENT, bias=bv_sb[:, mc:mc + 1])
                        # transpose VT tile [128, 576] -> V chunks [<=128, 128]
                        for c5 in range(5):
                            sz = 128 if c5 < 4 else 64
                            pt = psT.tile([128, 128], f32r, tag="pt")
                            nc.tensor.transpose(
                                pt[:sz, :], vo[:, c5 * 128: c5 * 128 + sz],
                                ident[:])
                            if half == 0:
                                ve = vev.tile([128, 256], f32r, tag=f"ve{c5}",
                                              name=f"ve{c5}")
                                ves.append(ve)
                            else:
                                ve = ves[c5]
                            nc.scalar.copy(
                                ve[:sz, half * 128:(half + 1) * 128],
                                pt[:sz, :])
                    for c5 in range(5):
                        sz = 128 if c5 < 4 else 64
                        nc.sync.dma_start(
                            V_d[c5 * 128: c5 * 128 + sz,
                                mcp * 256:(mcp + 1) * 256],
                            ves[c5][:sz, :])

            # ---------------- Phase B: QT_d = (q @ wq + bq).T ----------------
            WSTRIP = 1024
            with ExitStack() as ctx:
                qtp = ctx.enter_context(tc.tile_pool(name="qtp", bufs=1))
                wqp = ctx.enter_context(tc.tile_pool(name="wqp", bufs=2))
                qout = ctx.enter_context(tc.tile_pool(name="qout", bufs=3))
                psB = ctx.enter_context(tc.tile_pool(name="psB", bufs=3, space="PSUM"))

                for w in range(SQ // WSTRIP if "B" in phases else 0):
                    qts = qtp.tile([128, KC * WSTRIP], f32r, tag="qts")
                    for g in range(4):
                        gk = KC // 4
                        nc.sync.dma_start(
                            qts[:, g * gk * WSTRIP:(g + 1) * gk * WSTRIP]
                            .rearrange("p (kc s) -> p kc s", kc=gk),
                            qT[g * gk * 128:(g + 1) * gk * 128,
                               w * WSTRIP:(w + 1) * WSTRIP]
                            .rearrange("(kc p) s -> p kc s", p=128))
                    for mc in range(MC):
                        wqc = wqp.tile([128, KC * 128], f32r, tag="wqc")
                        nc.sync.dma_start(
                            wqc[:].rearrange("p (kc m) -> p kc m", kc=KC),
                            wq[:, mc * 128:(mc + 1) * 128]
                            .rearrange("(kc p) m -> p kc m", p=128))
                        ps = psB.tile([128, WSTRIP], f32, tag="psB")
                        for kc in range(KC):
                            lhs = wqc[:, kc * 128:(kc + 1) * 128]
                            for g in range(WSTRIP // 512):
                                nc.tensor.matmul(
                                    ps[:, g * 512:(g + 1) * 512], lhs,
                                    qts[:, kc * WSTRIP + g * 512:
                                        kc * WSTRIP + (g + 1) * 512],
                                    start=(kc == 0), stop=(kc == KC - 1))
                        qo = qout.tile([128, WSTRIP], f32r, tag="qo")
                        nc.scalar.activation(qo[:], ps[:], IDENT,
                                             bias=bq_sb[:, mc:mc + 1])
                        nc.sync.dma_start(
                            QT_d[mc * 128:(mc + 1) * 128,
                                 w * WSTRIP:(w + 1) * WSTRIP], qo[:])

            # ---------------- Phase C: attention per local head ----------------
            # ctxT stays resident across C and D: [128, 16*2048] = 128 KB/part
            ctxp = top.enter_context(tc.tile_pool(name="ctxp", bufs=1))
            ctx_all = ctxp.tile([128, MC * SQ], f32r)

            KEY_CH = [(0, 128), (128, 256), (256, 384), (384, 512), (512, 576)]
            with ExitStack() as ctx:
                khp = ctx.enter_context(tc.tile_pool(name="khp", bufs=2))
                vhp = ctx.enter_context(tc.tile_pool(name="vhp", bufs=2))
                qhp = ctx.enter_context(tc.tile_pool(name="qhp", bufs=3))
                stp = ctx.enter_context(tc.tile_pool(name="stp", bufs=2))
                rbp = ctx.enter_context(tc.tile_pool(name="rbp", bufs=2))
                aop = ctx.enter_context(tc.tile_pool(name="aop", bufs=3))
                rvp = ctx.enter_context(tc.tile_pool(name="rvp", bufs=2))
                psST = ctx.enter_context(tc.tile_pool(name="psST", bufs=2, space="PSUM"))
                psR = ctx.enter_context(tc.tile_pool(name="psR", bufs=2, space="PSUM"))
                psBC = ctx.enter_context(tc.tile_pool(name="psBC", bufs=2, space="PSUM"))
                psCX = ctx.enter_context(tc.tile_pool(name="psCX", bufs=2, space="PSUM"))

                for h in range(HLOC if "C" in phases else 0):
                    kth = khp.tile([128, 2 * SK], f32r, tag="kth")
                    nc.sync.dma_start(
                        kth[:].rearrange("p (c s) -> p c s", c=2),
                        KT_d[h * HD:(h + 1) * HD, :]
                        .rearrange("(c p) s -> p c s", p=128))
                    vha = vhp.tile([128, 4 * HD], f32r, tag="vha")
                    nc.sync.dma_start(
                        vha[:].rearrange("p (c d) -> p c d", c=4),
                        V_d[0:512, h * HD:(h + 1) * HD]
                        .rearrange("(c p) d -> p c d", p=128))
                    vhb = vhp.tile([64, HD], f32r, tag="vhb")
                    nc.sync.dma_start(vhb[:], V_d[512:SK, h * HD:(h + 1) * HD])

                    for rg in range(SQ // 512):
                        qth = qhp.tile([128, 2 * 512], f32r, tag="qth")
                        nc.sync.dma_start(
                            qth[:].rearrange("p (c s) -> p c s", c=2),
                            QT_d[h * HD:(h + 1) * HD, rg * 512:(rg + 1) * 512]
                            .rearrange("(c p) s -> p c s", p=128))

                        stn = stp.tile([128, 5 * 512], f32r, tag="stn")
                        for c5, (k0, k1) in enumerate(KEY_CH):
                            sz = k1 - k0
                            pst = psST.tile([128, 512], f32, tag="pst")
                            for c2 in range(2):
                                nc.tensor.matmul(
                                    pst[:sz, :],
                                    kth[:, c2 * SK + k0: c2 * SK + k1],
                                    qth[:, c2 * 512:(c2 + 1) * 512],
                                    start=(c2 == 0), stop=(c2 == 1))
                            nc.scalar.activation(
                                stn[:sz, c5 * 512:(c5 + 1) * 512], pst[:sz, :],
                                EXP, scale=scale)

                        # column sums over keys via ones-matmul, then 1/x
                        psr = psR.tile([1, 512], f32, tag="psr")
                        for c5, (k0, k1) in enumerate(KEY_CH):
                            sz = k1 - k0
                            nc.tensor.matmul(
                                psr[:, :], ones_col[:sz, :],
                                stn[:sz, c5 * 512:(c5 + 1) * 512],
                                start=(c5 == 0), stop=(c5 == 4))
                        rinv = rvp.tile([1, 512], f32r, tag="rinv")
                        with nc.allow_low_precision(
                                reason="softmax 1/sum consumed by f32r matmul"):
                            nc.vector.reciprocal(rinv[:], psr[:, :])
                        # broadcast rinv across 128 partitions: ones [1,128]^T @ rinv
                        psb = psBC.tile([128, 512], f32, tag="psb")
                        nc.tensor.matmul(psb[:], ones_row[:],
                                         rinv[:],
                                         start=True, stop=True)
                        rb = rbp.tile([128, 512], f32, tag="rb")
                        nc.scalar.copy(rb[:], psb[:])

                        for c5, (k0, k1) in enumerate(KEY_CH):
                            sz = k1 - k0
                            sl = stn[:sz, c5 * 512:(c5 + 1) * 512]
                            aout = aop.tile([128, 512], f32, tag="aout")
                            nc.vector.tensor_mul(aout[:sz, :], sl, rb[:sz, :])
                            nc.sync.dma_start(
                                attn_p[h, k0:k1, rg * 512:(rg + 1) * 512],
                                aout[:sz, :])

                        for c2 in range(2):
                            pcx = psCX.tile([128, 512], f32, tag="pcx")
                            for c5, (k0, k1) in enumerate(KEY_CH):
                                sz = k1 - k0
                                if c5 < 4:
                                    vsl = vha[:, c5 * HD + c2 * 128:
                                              c5 * HD + (c2 + 1) * 128]
                                else:
                                    vsl = vhb[:, c2 * 128:(c2 + 1) * 128]
                                nc.tensor.matmul(
                                    pcx[:], vsl,
                                    stn[:sz, c5 * 512:(c5 + 1) * 512],
                                    start=(c5 == 0), stop=(c5 == 4))
                            nc.vector.tensor_mul(
                                ctx_all[:, (h * 2 + c2) * SQ + rg * 512:
                                        (h * 2 + c2) * SQ + (rg + 1) * 512],
                                pcx[:], rb[:])

            # ---------------- Phase D: outT_p = wo.T @ ctxT + bo/2 ----------------
            with ExitStack() as ctx:
                wop = ctx.enter_context(tc.tile_pool(name="wop", bufs=3))
                opool = ctx.enter_context(tc.tile_pool(name="opool", bufs=4))
                psD = ctx.enter_context(tc.tile_pool(name="psD", bufs=2, space="PSUM"))

                for oc in range(D // 128 if "D" in phases else 0):
                    woc = wop.tile([128, MC * 128], f32r, tag="woc")
                    nc.sync.dma_start(
                        woc[:].rearrange("p (fc m) -> p fc m", fc=MC),
                        wo[:, oc * 128:(oc + 1) * 128]
                        .rearrange("(fc p) m -> p fc m", p=128))
                    pss = [psD.tile([128, 512], f32, tag=f"psD{rg}",
                                    name=f"psD{rg}") for rg in range(4)]
                    for fc in range(MC):
                        lhs = woc[:, fc * 128:(fc + 1) * 128]
                        for rg in range(4):
                            nc.tensor.matmul(
                                pss[rg][:], lhs,
                                ctx_all[:, fc * SQ + rg * 512:
                                        fc * SQ + (rg + 1) * 512],
                                start=(fc == 0), stop=(fc == MC - 1))
                    for rg in range(4):
                        osb = opool.tile([128, 512], f32, tag="osb")
                        nc.scalar.activation(osb[:], pss[rg][:], IDENT,
                                             bias=bo2_sb[:, oc:oc + 1])
                        nc.sync.dma_start(
                            outT_p[oc * 128:(oc + 1) * 128,
                                   rg * 512:(rg + 1) * 512], osb[:])

    nc.compile()
    _BUILD_CACHE[key] = nc
    return nc


def _make_in_maps(query, key, value, Wq, bq, Wk, bk, Wv, bv, Wo, bo):
    f = np.float32
    bo2 = (np.asarray(bo, f) / 2.0).astype(f)
    in_maps = []
    for c in range(N_CORES):
        b, s = c // 2, c % 2
        sl = slice(s * DLOC, (s + 1) * DLOC)
        in_maps.append({
            "qT": np.ascontiguousarray(np.asarray(query[b], f).T),
            "kT": np.ascontiguousarray(np.asarray(key[b], f).T),
            "vT": np.ascontiguousarray(np.asarray(value[b], f).T),
            "wq": np.ascontiguousarray(np.asarray(Wq, f)[:, sl]),
            "wk": np.ascontiguousarray(np.asarray(Wk, f)[:, sl]),
            "wv": np.ascontiguousarray(np.asarray(Wv, f)[:, sl]),
            "wo": np.ascontiguousarray(np.asarray(Wo, f)[sl, :]),
            "bq": np.ascontiguousarray(np.asarray(bq, f)[sl]),
            "bk": np.ascontiguousarray(np.asarray(bk, f)[sl]),
            "bv": np.ascontiguousarray(np.asarray(bv, f)[sl]),
            "bo2": bo2,
        })
    return in_maps


def _gather(results):
    out = np.empty((B, SQ, D), np.float32)
    attn = np.empty((B, SQ, SK), np.float32)
    for b in range(B):
        r0, r1 = results[2 * b], results[2 * b + 1]
        out[b] = (r0["outT_p"] + r1["outT_p"]).T
        attn[b] = ((r0["attn_p"].sum(axis=0) + r1["attn_p"].sum(axis=0)) / H).T
    return out, attn


def _enable_jax_compile_cache():
    try:
        import jax
        jax.config.update("jax_compilation_cache_dir",
                          "/tmp/jax_neff_cache_cma74002286510558")
        jax.config.update("jax_persistent_cache_min_compile_time_secs", 0.0)
        jax.config.update("jax_persistent_cache_min_entry_size_bytes", -1)
    except Exception:
        pass


def kernel(query, key, value, Wq, bq, Wk, bk, Wv, bv, Wo, bo, temperature):
    from concourse.bass_utils import run_bass_kernel_spmd

    _enable_jax_compile_cache()

    temp = float(np.asarray(temperature))
    nc = _build(SCALE / temp)
    in_maps = _make_in_maps(query, key, value, Wq, bq, Wk, bk, Wv, bv, Wo, bo)
    res = run_bass_kernel_spmd(nc, in_maps, core_ids=list(range(N_CORES)))
    return _gather(res.results)
